# revision 9
# baseline (speedup 1.0000x reference)
"""DGINConv (2-layer GIN with edge features) Trainium2 kernel.

Math (per layer, reference):
    h_node = x @ W_node.T
    h_edge[i,j,:] = W_edge @ edges[i,j,:]
    ne = relu(h_node[None,:,:] + h_edge + bne)             # [N, N, din]
    msg = einsum('ij,ijd->id', adj, ne)
    out = relu(((1+eps)*x + msg) @ Wn.T + bn)

Algebraic restructure (requires adj in {0,1}):
    adj*relu(hb + he) = relu(hb + adj*he) - relu(hb) + adj*relu(hb)
With me := adj-masked edges (he is linear in edges):
    msg[i,d] = sum_j relu(hb[j,d] + mhe[i,j,d])      (term1, the big one)
             - sum_j relu(hb[j,d])                   (term2: per-layer const)
             + (adj @ relu(hb))[i,d]                 (term3: one small matmul)
The hb representation error cancels exactly between term1/term2 for
inactive pairs (masked-edge columns are exactly zero in PSUM).

Distribution: destination rows sharded 8 ways; nodes/weights replicated;
one AllGather of updated node features between layers.

term1 dataflow per own-row i (PSUM-exit bound — both vector engines split
the work):  PSUM[i] [128 d, 1024 j] (2 banks)
  = 4x row-tiled K=32 matmul of transposed masked edges (PE, tile_position)
  (+ Identity-matmul of hbT for ACT-consumed rows only)
  -> ACT rows: activation(Relu, accum_out)      = sum_j relu(hb + mhe)
  -> DVE rows: custom DVE op relu(in0+in1) with accumulate (hb added from
     SBUF by the op itself, no Identity-matmul needed).

me layout [128p=(t,k), g, jt, j]: xbar DMA-transpose of the natural-layout
bf16 masked edges; row-group 32t holds edges^T (k-major) of own row 4g+t,
feeding the 4 concurrent row-tiled matmuls directly.
"""

import sys

if "/opt/trn_rl_repo" not in sys.path:
    sys.path.insert(0, "/opt/trn_rl_repo")

import numpy as np

N, D, E, NC = 1024, 128, 32, 8
SH = N // NC          # 128 rows per core
NG = SH // 4          # 32 groups of 4 own-rows
NJT = N // 128        # 8 j-tiles
JH = 512
ACT_PER16 = 8         # of every 16 groups, this many consumed by ACT
_cache = {}

_CUSTOM = {}


def _ensure_custom_op():
    """Register RELU_ADD_REDUCE_GIN: out = relu(in0 + in1); accum = sum."""
    if "op" in _CUSTOM:
        return _CUSTOM["op"]
    import concourse.dve_ops as dve_ops
    from concourse.dve_spec import Spec, Src0, Src1, relu, lower, _has_src1
    from concourse.dve_uop import DveOpSpec
    from operator import add

    name = "RELU_ADD_REDUCE_GIN"

    def _ref(in0, in1, c0, c1, c2):
        b = dve_ops._dve_relu(in0.astype(np.float32) + in1.astype(np.float32))
        return b, b.reshape(b.shape[0], -1).sum(axis=-1, keepdims=True)

    from concourse.dve_spec import Zero

    spec = Spec(body=relu(Src0 + Src1), accum=add, accum_init=Zero,
                reference=_ref)
    row = dve_ops._CUSTOM_DVE_ROW_BASE + len(dve_ops.OPS)
    assert row < 0x20
    shas = {}
    for ver in ("v3", "v4"):
        try:
            s = DveOpSpec(name=name, opcode=row, uops=lower(spec, ver=ver),
                          rd1_en=_has_src1(spec))
            shas[ver] = s.sha(ver)
        except Exception:
            pass
    op = dve_ops.DveOp(name, spec, subdim=False, uops_sha=shas)
    dve_ops.OPS.append(op)
    dve_ops.CUSTOM_DVE_SPECS[name] = spec
    dve_ops._SUB_OPCODE_FOR_NAME[name] = row
    _CUSTOM["op"] = op
    return op


def _build_nc(mode="full"):
    from contextlib import ExitStack

    import concourse.mybir as mybir
    import concourse.tile as tile
    from concourse import bacc
    from concourse.masks import make_identity

    relu_add_reduce = _ensure_custom_op()

    f32 = mybir.dt.float32
    bf16 = mybir.dt.bfloat16
    RELU = mybir.ActivationFunctionType.Relu
    ADD = mybir.AluOpType.add
    MULT = mybir.AluOpType.mult
    SUB = mybir.AluOpType.subtract
    MAX = mybir.AluOpType.max

    nc = bacc.Bacc("TRN2", target_bir_lowering=False, debug=False,
                   enable_asserts=False, num_devices=NC)

    def din(name, shape):
        return nc.dram_tensor(name, shape, f32, kind="ExternalInput").ap()

    edges_d = din("edges_sh", [SH, N, E])
    adjT_d = din("adjT_sh", [N, SH])        # host: adj[own].T
    xT_d = din("xT", [D, N])                # host: nodes.T
    xownT_d = din("x_ownT", [D, SH])        # host: nodes[own].T
    opse_d = din("opse", [128, 1])          # host: 1+eps replicated
    WeT_d = [din(f"WeT{l}", [E, D]) for l in range(2)]
    WnodeT_d = [din(f"WnodeT{l}", [D, D]) for l in range(2)]
    WnT_d = [din(f"WnT{l}", [D, D]) for l in range(2)]
    bne_d = [din(f"bne{l}", [D, 1]) for l in range(2)]
    bn_d = [din(f"bn{l}", [D, 1]) for l in range(2)]
    out_d = nc.dram_tensor("out", [SH, D], f32, kind="ExternalOutput").ap()

    with tile.TileContext(nc) as tc, ExitStack() as ctx:
        P = ctx.enter_context(tc.tile_pool(name="persist", bufs=1))
        dramp = ctx.enter_context(tc.tile_pool(name="dram", bufs=1, space="DRAM"))
        natp = ctx.enter_context(tc.tile_pool(name="nat", bufs=3))
        psumM = ctx.enter_context(tc.tile_pool(name="psumM", bufs=3, space="PSUM"))
        psumS = ctx.enter_context(tc.tile_pool(name="psumS", bufs=1, space="PSUM"))
        scrp = ctx.enter_context(tc.tile_pool(name="scr", bufs=4))

        # ---------------- constants / small inputs ----------------
        ident = P.tile([128, 128], bf16)
        make_identity(nc, ident[:])

        opse = P.tile([128, 1], f32)
        nc.sync.dma_start(out=opse[:], in_=opse_d[:])
        xownT = P.tile([D, SH], f32)
        nc.sync.dma_start(out=xownT[:], in_=xownT_d[:])

        WeT_rep, WnodeT, WnT, bne, bn = [], [], [], [], []
        for l in range(2):
            w = P.tile([128, 128], bf16, tag=f"WeTrep{l}")
            for t in range(4):
                nc.gpsimd.dma_start(out=w[32 * t:32 * (t + 1), :], in_=WeT_d[l][:])
            WeT_rep.append(w)
            wn = P.tile([D, D], bf16, tag=f"WnodeT{l}")
            nc.gpsimd.dma_start(out=wn[:], in_=WnodeT_d[l][:])
            WnodeT.append(wn)
            wo = P.tile([D, D], bf16, tag=f"WnT{l}")
            nc.gpsimd.dma_start(out=wo[:], in_=WnT_d[l][:])
            WnT.append(wo)
            b0 = P.tile([D, 1], f32, tag=f"bne{l}")
            nc.sync.dma_start(out=b0[:], in_=bne_d[l][:])
            bne.append(b0)
            b1 = P.tile([D, 1], f32, tag=f"bn{l}")
            nc.sync.dma_start(out=b1[:], in_=bn_d[l][:])
            bn.append(b1)

        # adjT: [jp, jt, i] bf16 (cast on DMA; host already transposed)
        adjT = P.tile([128, NJT, SH], bf16)
        nc.gpsimd.dma_start(
            out=adjT[:], in_=adjT_d.rearrange("(jt p) i -> p jt i", p=128))

        # xT layer 0: [c, j] bf16
        xT0 = P.tile([D, N], bf16)
        nc.gpsimd.dma_start(out=xT0[:], in_=xT_d[:])

        # ------------- edges: load(f32) + mask*cast + transpose -------------
        me = P.tile([128, NG, NJT, 128], bf16)
        for g in range(NG):
            natf = natp.tile([128, NJT, 4, E], f32, tag="natf")
            for t in range(4):
                nc.sync.dma_start(
                    out=natf[:, :, t, :],
                    in_=edges_d[4 * g + t].rearrange("(jt p) k -> p jt k", p=128))
            natb = natp.tile([128, NJT, 4, E], bf16, tag="natb")
            adj_bc = adjT[:, :, 4 * g:4 * (g + 1)].unsqueeze(3).broadcast_to(
                [128, NJT, 4, E])
            nc.vector.tensor_tensor(out=natb[:], in0=natf[:], in1=adj_bc, op=MULT)
            for jt in range(NJT):
                nc.sync.dma_start(out=me[:, g, jt, :], in_=natb[:, jt, :, :],
                                  transpose=True)

        # ---------------- per-layer helpers ----------------
        def hb_prep(l, xT_l):
            hbT = P.tile([D, N], bf16, tag=f"hbT{l}")
            for h in range(2):
                ps = psumS.tile([128, JH], f32, tag="small")
                nc.tensor.matmul(out=ps[:], lhsT=WnodeT[l][:],
                                 rhs=xT_l[:, h * JH:(h + 1) * JH],
                                 start=True, stop=True)
                nc.vector.tensor_scalar(out=hbT[:, h * JH:(h + 1) * JH],
                                        in0=ps[:], scalar1=bne[l][:],
                                        scalar2=None, op0=ADD)
            srh_p = P.tile([D, 2], f32, tag=f"srhp{l}")
            for h in range(2):
                scr = scrp.tile([128, 2 * JH], bf16, tag="scr_main")
                nc.scalar.activation(out=scr[:, 0:JH],
                                     in_=hbT[:, h * JH:(h + 1) * JH],
                                     func=RELU, accum_out=srh_p[:, h:h + 1])
            srh = P.tile([D, 1], f32, tag=f"srh{l}")
            nc.vector.tensor_tensor(out=srh[:], in0=srh_p[:, 0:1],
                                    in1=srh_p[:, 1:2], op=ADD)
            rhb_raw = P.tile([128, NJT, D], bf16, tag=f"rhbraw{l}")
            for jt in range(NJT):
                nc.sync.dma_start(out=rhb_raw[:, jt, :],
                                  in_=hbT[:, jt * 128:(jt + 1) * 128],
                                  transpose=True)
            reluhb = P.tile([128, NJT, D], bf16, tag=f"reluhb{l}")
            nc.vector.tensor_scalar(out=reluhb[:], in0=rhb_raw[:],
                                    scalar1=0.0, scalar2=None, op0=MAX)
            return hbT, srh, reluhb

        def layer(l, xT_l, xownT_l):
            hbT, srh, reluhb = hb_prep(l, xT_l)

            # term1: msgparts[d, i_own]
            msgparts = P.tile([D, SH], f32, tag=f"msgp{l}")
            for g in range(NG):
                use_act = (g % 16) < ACT_PER16
                pss = []
                for t in range(4):
                    ps = psumM.tile([128, 2 * JH], f32, tag="ps_main")
                    pss.append(ps)
                if use_act:
                    for t in range(4):
                        for h in range(2):
                            nc.tensor.matmul(
                                out=pss[t][:, h * JH:(h + 1) * JH],
                                lhsT=ident[:],
                                rhs=hbT[:, h * JH:(h + 1) * JH],
                                start=True, stop=False)
                for t in range(4):
                    for h in range(2):
                        nc.tensor.matmul(
                            out=pss[t][:, h * JH:(h + 1) * JH],
                            lhsT=WeT_rep[l][32 * t:32 * (t + 1), :],
                            rhs=me[32 * t:32 * (t + 1), g, 4 * h:4 * (h + 1), :],
                            start=not use_act, stop=True,
                            tile_position=(32 * t, 0))
                for t in range(4):
                    idx = 4 * g + t
                    scr = scrp.tile([128, 2 * JH], bf16, tag="scr_main")
                    if use_act:
                        nc.scalar.activation(
                            out=scr[:], in_=pss[t][:], func=RELU,
                            accum_out=msgparts[:, idx:idx + 1])
                    else:
                        nc.vector._custom_dve(
                            relu_add_reduce, out=scr[:], in0=pss[t][:],
                            in1=hbT[:], accum_out=msgparts[:, idx:idx + 1])

            # term3: corr[d, i] = sum_j reluhb[j, d] * adjT[j, i]
            ps_corr = psumS.tile([D, SH], f32, tag="small")
            for jt in range(NJT):
                nc.tensor.matmul(out=ps_corr[:], lhsT=reluhb[:, jt, :],
                                 rhs=adjT[:, jt, :],
                                 start=(jt == 0), stop=(jt == NJT - 1))

            # z = (1+eps)*x_own + msg   (all [d, i_own])
            zt = P.tile([D, SH], f32, tag=f"zt{l}")
            nc.vector.scalar_tensor_tensor(out=zt[:], in0=msgparts[:],
                                           scalar=srh[:], in1=ps_corr[:],
                                           op0=SUB, op1=ADD)
            z_bf = P.tile([D, SH], bf16, tag=f"zbf{l}")
            nc.vector.scalar_tensor_tensor(out=z_bf[:], in0=xownT_l[:],
                                           scalar=opse[:], in1=zt[:],
                                           op0=MULT, op1=ADD)

            # h = relu(Wn @ z + bn) -> [d_out, i_own]
            ps_h = psumS.tile([D, SH], f32, tag="small")
            nc.tensor.matmul(out=ps_h[:], lhsT=WnT[l][:], rhs=z_bf[:],
                             start=True, stop=True)
            hT = P.tile([D, SH], f32, tag=f"hT{l}")
            nc.scalar.activation(out=hT[:], in_=ps_h[:], func=RELU, bias=bn[l][:])
            return hT

        # ---------------- layer 0 ----------------
        if mode == "edges":
            ps_e = psumS.tile([D, SH], f32, tag="small")
            nc.tensor.matmul(out=ps_e[:], lhsT=ident[:], rhs=me[:, 0, 0, :],
                             start=True, stop=True)
            h2T = P.tile([D, SH], f32, tag="hTdbg")
            nc.scalar.copy(h2T[:], ps_e[:])
        else:
            h1T = layer(0, xT0, xownT)

        if mode == "l1":
            h2T = h1T
        elif mode == "nocc":
            h2T = layer(1, xT0, h1T)
        elif mode == "full":
            # ------------- allgather updated node features -------------
            h1T_bf = P.tile([D, SH], bf16)
            nc.vector.tensor_scalar(out=h1T_bf[:], in0=h1T[:], scalar1=0.0,
                                    scalar2=None, op0=ADD)
            ps_t = psumS.tile([SH, D], bf16, tag="small")
            nc.tensor.transpose(ps_t[:], h1T_bf[:], ident[:])
            h1_own = P.tile([SH, D], f32)
            nc.scalar.copy(h1_own[:], ps_t[:])

            gin = dramp.tile([SH, D], f32)
            gout = dramp.tile([N, D], f32)
            nc.gpsimd.dma_start(out=gin[:], in_=h1_own[:])
            nc.gpsimd.collective_compute(
                "AllGather", mybir.AluOpType.bypass,
                replica_groups=[list(range(NC))],
                ins=[gin[:].opt()], outs=[gout[:].opt()])

            # x1T [c, j] bf16 from gathered [N, D] f32: cast + xbar-transpose
            x1b = P.tile([128, NJT, D], bf16)
            nc.gpsimd.dma_start(
                out=x1b[:], in_=gout[:].rearrange("(jt p) d -> p jt d", p=128))
            x1T = P.tile([D, N], bf16)
            for jt in range(NJT):
                nc.sync.dma_start(out=x1T[:, jt * 128:(jt + 1) * 128],
                                  in_=x1b[:, jt, :], transpose=True)

            # ---------------- layer 1 ----------------
            h2T = layer(1, x1T, h1T)

        # ---------------- output ----------------
        h2T_bf = P.tile([D, SH], bf16)
        nc.vector.tensor_scalar(out=h2T_bf[:], in0=h2T[:], scalar1=0.0,
                                scalar2=None, op0=ADD)
        ps_o = psumS.tile([SH, D], bf16, tag="small")
        nc.tensor.transpose(ps_o[:], h2T_bf[:], ident[:])
        h2_own = P.tile([SH, D], f32)
        nc.scalar.copy(h2_own[:], ps_o[:])
        nc.sync.dma_start(out=out_d[:], in_=h2_own[:])

    nc.compile()
    return nc


def _host_inputs(inputs):
    """Build the 8 per-core input maps from full inputs (host-side staging)."""
    adj = np.ascontiguousarray(np.asarray(inputs["adj"], np.float32))
    nodes = np.ascontiguousarray(np.asarray(inputs["nodes"], np.float32))
    edges = np.ascontiguousarray(np.asarray(inputs["edges"], np.float32))
    eps = float(np.asarray(inputs["eps"], np.float32).reshape(-1)[0])
    com = {
        "xT": np.ascontiguousarray(nodes.T),
        "opse": np.full((128, 1), 1.0 + eps, np.float32),
    }
    Wne = [np.asarray(inputs["Wne0"], np.float32),
           np.asarray(inputs["Wne1"], np.float32)]
    for l in range(2):
        com[f"WeT{l}"] = np.ascontiguousarray(Wne[l][:, D:D + E].T)
        com[f"WnodeT{l}"] = np.ascontiguousarray(Wne[l][:, :D].T)
        com[f"WnT{l}"] = np.ascontiguousarray(
            np.asarray(inputs[f"Wn{l}"], np.float32).T)
        com[f"bne{l}"] = np.ascontiguousarray(
            np.asarray(inputs[f"bne{l}"], np.float32).reshape(D, 1))
        com[f"bn{l}"] = np.ascontiguousarray(
            np.asarray(inputs[f"bn{l}"], np.float32).reshape(D, 1))
    maps = []
    for c in range(NC):
        sl = slice(SH * c, SH * (c + 1))
        m = dict(com)
        m["edges_sh"] = edges[sl]
        m["adjT_sh"] = np.ascontiguousarray(adj[sl].T)
        m["x_ownT"] = np.ascontiguousarray(nodes[sl].T)
        maps.append(m)
    return maps


def _get_runner():
    """Build (once) a cached jit(shard_map) callable over the compiled module."""
    if "runner" in _cache:
        return _cache["runner"]
    import jax
    from jax.sharding import Mesh, PartitionSpec, NamedSharding
    from jax.experimental.shard_map import shard_map
    import concourse.mybir as mybir
    from concourse import bass2jax
    from concourse.bass2jax import _bass_exec_p, partition_id_tensor

    if "nc" not in _cache:
        _cache["nc"] = _build_nc()
    nc = _cache["nc"]
    bass2jax.install_neuronx_cc_hook()

    in_names, out_names, out_avals, zero_outs = [], [], [], []
    partition_name = nc.partition_id_tensor.name if nc.partition_id_tensor else None
    for alloc in nc.m.functions[0].allocations:
        if not isinstance(alloc, mybir.MemoryLocationSet):
            continue
        name = alloc.memorylocations[0].name
        if alloc.kind == "ExternalInput":
            if name != partition_name:
                in_names.append(name)
        elif alloc.kind == "ExternalOutput":
            shape = list(alloc.tensor_shape)
            dtype = np.dtype(mybir.dt.np(alloc.dtype))
            out_avals.append(jax.core.ShapedArray(shape, dtype))
            out_names.append(name)
            zero_outs.append(np.zeros(shape, dtype))

    n_params = len(in_names)
    all_in_names = list(in_names) + list(out_names)
    if partition_name is not None:
        all_in_names.append(partition_name)

    def _body(*args):
        operands = list(args)
        if partition_name is not None:
            operands.append(partition_id_tensor())
        outs = _bass_exec_p.bind(
            *operands,
            out_avals=tuple(out_avals),
            in_names=tuple(all_in_names),
            out_names=tuple(out_names),
            lowering_input_output_aliases=(),
            sim_require_finite=True,
            sim_require_nnan=True,
            nc=nc,
        )
        return tuple(outs)

    devices = jax.devices()[:NC]
    mesh = Mesh(np.asarray(devices), ("core",))
    n_outs = len(out_names)
    fn = jax.jit(
        shard_map(_body, mesh=mesh,
                  in_specs=(PartitionSpec("core"),) * (n_params + n_outs),
                  out_specs=(PartitionSpec("core"),) * n_outs,
                  check_rep=False),
        keep_unused=True)
    sh = NamedSharding(mesh, PartitionSpec("core"))
    dev_zeros = [
        jax.device_put(np.zeros((NC * z.shape[0], *z.shape[1:]), z.dtype), sh)
        for z in zero_outs
    ]

    def run(maps):
        dev_in = []
        for nm in in_names:
            arrs = [
                jax.device_put(np.asarray(maps[c][nm]), devices[c])
                for c in range(NC)
            ]
            shp = arrs[0].shape
            glob = jax.make_array_from_single_device_arrays(
                (NC * shp[0], *shp[1:]), sh, arrs)
            dev_in.append(glob)
        outs = fn(*dev_in, *dev_zeros)
        oi = out_names.index("out")
        return np.asarray(outs[oi]).reshape(NC, SH, D).reshape(N, D)

    _cache["runner"] = run
    return run


def kernel(**inputs):
    run = _get_runner()
    maps = _host_inputs(inputs)
    return np.ascontiguousarray(run(maps).astype(np.float32))


if __name__ == "__main__":
    _build_nc()
    print("build+compile OK")


# revision 13
# speedup vs baseline: 42.4667x; 42.4667x over previous
"""DGINConv (2-layer GIN with edge features) Trainium2 kernel.

Math (per layer, reference):
    h_node = x @ W_node.T
    h_edge[i,j,:] = W_edge @ edges[i,j,:]
    ne = relu(h_node[None,:,:] + h_edge + bne)             # [N, N, din]
    msg = einsum('ij,ijd->id', adj, ne)
    out = relu(((1+eps)*x + msg) @ Wn.T + bn)

Algebraic restructure (requires adj in {0,1}):
    adj*relu(hb + he) = relu(hb + adj*he) - relu(hb) + adj*relu(hb)
With me := adj-masked edges (he is linear in edges):
    msg[i,d] = sum_j relu(hb[j,d] + mhe[i,j,d])      (term1, the big one)
             - sum_j relu(hb[j,d])                   (term2: per-layer const)
             + (adj @ relu(hb))[i,d]                 (term3: one small matmul)
The hb representation error cancels exactly between term1/term2 for
inactive pairs (masked-edge columns are exactly zero in PSUM).

Distribution: destination rows sharded 8 ways; nodes/weights replicated;
one AllGather of updated node features between layers.

term1 dataflow per own-row i (PSUM-exit bound — both vector engines split
the work):  PSUM[i] [128 d, 1024 j] (2 banks)
  = 4x row-tiled K=32 matmul of transposed masked edges (PE, tile_position)
  (+ Identity-matmul of hbT for ACT-consumed rows only)
  -> ACT rows: activation(Relu, accum_out)      = sum_j relu(hb + mhe)
  -> DVE rows: custom DVE op relu(in0+in1) with accumulate (hb added from
     SBUF by the op itself, no Identity-matmul needed).

me layout [128p=(t,k), g, jt, j]: xbar DMA-transpose of the natural-layout
bf16 masked edges; row-group 32t holds edges^T (k-major) of own row 4g+t,
feeding the 4 concurrent row-tiled matmuls directly.
"""

import sys

if "/opt/trn_rl_repo" not in sys.path:
    sys.path.insert(0, "/opt/trn_rl_repo")

import numpy as np

N, D, E, NC = 1024, 128, 32, 8
SH = N // NC          # 128 rows per core
NG = SH // 4          # 32 groups of 4 own-rows
NJT = N // 128        # 8 j-tiles
JH = 512
ACT_PER16 = 8         # of every 16 groups, this many consumed by ACT
_cache = {}

_CUSTOM = {}


def _ensure_custom_op():
    """Register RELU_ADD_REDUCE_GIN: out = relu(in0 + in1); accum = sum."""
    if "op" in _CUSTOM:
        return _CUSTOM["op"]
    import concourse.dve_ops as dve_ops
    from concourse.dve_spec import Spec, Src0, Src1, relu, lower, _has_src1
    from concourse.dve_uop import DveOpSpec
    from operator import add

    name = "RELU_ADD_REDUCE_GIN"

    def _ref(in0, in1, c0, c1, c2):
        b = dve_ops._dve_relu(in0.astype(np.float32) + in1.astype(np.float32))
        return b, b.reshape(b.shape[0], -1).sum(axis=-1, keepdims=True)

    from concourse.dve_spec import Zero

    spec = Spec(body=relu(Src0 + Src1), accum=add, accum_init=Zero,
                reference=_ref)
    row = dve_ops._CUSTOM_DVE_ROW_BASE + len(dve_ops.OPS)
    assert row < 0x20
    shas = {}
    for ver in ("v3", "v4"):
        try:
            s = DveOpSpec(name=name, opcode=row, uops=lower(spec, ver=ver),
                          rd1_en=_has_src1(spec))
            shas[ver] = s.sha(ver)
        except Exception:
            pass
    op = dve_ops.DveOp(name, spec, subdim=False, uops_sha=shas)
    dve_ops.OPS.append(op)
    dve_ops.CUSTOM_DVE_SPECS[name] = spec
    dve_ops._SUB_OPCODE_FOR_NAME[name] = row
    _CUSTOM["op"] = op
    return op


def _build_nc(mode="full"):
    from contextlib import ExitStack

    import concourse.mybir as mybir
    import concourse.tile as tile
    from concourse import bacc
    from concourse.masks import make_identity

    relu_add_reduce = _ensure_custom_op()

    f32 = mybir.dt.float32
    bf16 = mybir.dt.bfloat16
    RELU = mybir.ActivationFunctionType.Relu
    ADD = mybir.AluOpType.add
    MULT = mybir.AluOpType.mult
    SUB = mybir.AluOpType.subtract
    MAX = mybir.AluOpType.max

    nc = bacc.Bacc("TRN2", target_bir_lowering=False, debug=False,
                   enable_asserts=False, num_devices=NC)

    def din(name, shape):
        return nc.dram_tensor(name, shape, f32, kind="ExternalInput").ap()

    edges_d = din("edges_sh", [SH, N, E])
    adjT_d = din("adjT_sh", [N, SH])        # host: adj[own].T
    xT_d = din("xT", [D, N])                # host: nodes.T
    xownT_d = din("x_ownT", [D, SH])        # host: nodes[own].T
    opse_d = din("opse", [128, 1])          # host: 1+eps replicated
    WeT_d = [din(f"WeT{l}", [E, D]) for l in range(2)]
    WnodeT_d = [din(f"WnodeT{l}", [D, D]) for l in range(2)]
    WnT_d = [din(f"WnT{l}", [D, D]) for l in range(2)]
    bne_d = [din(f"bne{l}", [D, 1]) for l in range(2)]
    bn_d = [din(f"bn{l}", [D, 1]) for l in range(2)]
    out_d = nc.dram_tensor("out", [SH, D], f32, kind="ExternalOutput").ap()

    with tile.TileContext(nc) as tc, ExitStack() as ctx:
        P = ctx.enter_context(tc.tile_pool(name="persist", bufs=1))
        dramp = ctx.enter_context(tc.tile_pool(name="dram", bufs=1, space="DRAM"))
        natp = ctx.enter_context(tc.tile_pool(name="nat", bufs=3))
        psumA = ctx.enter_context(tc.tile_pool(name="psumA", bufs=2, space="PSUM"))
        psumD = ctx.enter_context(tc.tile_pool(name="psumD", bufs=2, space="PSUM"))
        scrp = ctx.enter_context(tc.tile_pool(name="scr", bufs=2))
        scrd = ctx.enter_context(tc.tile_pool(name="scrd", bufs=2))

        # ---------------- constants / small inputs ----------------
        ident = P.tile([128, 128], bf16)
        make_identity(nc, ident[:])

        opse = P.tile([128, 1], f32)
        nc.sync.dma_start(out=opse[:], in_=opse_d[:])
        xownT = P.tile([D, SH], f32)
        nc.sync.dma_start(out=xownT[:], in_=xownT_d[:])

        WeT_rep, WnodeT, WnT, bne, bn = [], [], [], [], []
        for l in range(2):
            w = P.tile([128, 128], bf16, tag=f"WeTrep{l}")
            for t in range(4):
                nc.gpsimd.dma_start(out=w[32 * t:32 * (t + 1), :], in_=WeT_d[l][:])
            WeT_rep.append(w)
            wn = P.tile([D, D], bf16, tag=f"WnodeT{l}")
            nc.gpsimd.dma_start(out=wn[:], in_=WnodeT_d[l][:])
            WnodeT.append(wn)
            wo = P.tile([D, D], bf16, tag=f"WnT{l}")
            nc.gpsimd.dma_start(out=wo[:], in_=WnT_d[l][:])
            WnT.append(wo)
            b0 = P.tile([D, 1], f32, tag=f"bne{l}")
            nc.sync.dma_start(out=b0[:], in_=bne_d[l][:])
            bne.append(b0)
            b1 = P.tile([D, 1], f32, tag=f"bn{l}")
            nc.sync.dma_start(out=b1[:], in_=bn_d[l][:])
            bn.append(b1)

        # adjT: [jp, jt, i] bf16 (cast on DMA; host already transposed)
        adjT = P.tile([128, NJT, SH], bf16)
        nc.gpsimd.dma_start(
            out=adjT[:], in_=adjT_d.rearrange("(jt p) i -> p jt i", p=128))

        # xT layer 0: [c, j] bf16
        xT0 = P.tile([D, N], bf16)
        nc.gpsimd.dma_start(out=xT0[:], in_=xT_d[:])

        # ------------- edges: load(f32) + mask*cast + transpose -------------
        me = P.tile([128, NG, NJT, 128], bf16)
        for g in range(NG):
            natf = natp.tile([128, NJT, 4, E], f32, tag="natf")
            for t in range(4):
                nc.scalar.dma_start(
                    out=natf[:, :, t, :],
                    in_=edges_d[4 * g + t].rearrange("(jt p) k -> p jt k", p=128))
            natb = natp.tile([128, NJT, 4, E], bf16, tag="natb")
            adj_bc = adjT[:, :, 4 * g:4 * (g + 1)].unsqueeze(3).broadcast_to(
                [128, NJT, 4, E])
            nc.gpsimd.tensor_tensor(out=natb[:], in0=natf[:], in1=adj_bc, op=MULT)
            for jt in range(NJT):
                nc.sync.dma_start(out=me[:, g, jt, :], in_=natb[:, jt, :, :],
                                  transpose=True)

        # ---------------- per-layer helpers ----------------
        def hb_prep(l, xT_l):
            hbT = P.tile([D, N], bf16, tag=f"hbT{l}")
            for h in range(2):
                ps = psumA.tile([128, JH], f32, tag="actps")
                nc.tensor.matmul(out=ps[:], lhsT=WnodeT[l][:],
                                 rhs=xT_l[:, h * JH:(h + 1) * JH],
                                 start=True, stop=True)
                nc.vector.tensor_scalar(out=hbT[:, h * JH:(h + 1) * JH],
                                        in0=ps[:], scalar1=bne[l][:],
                                        scalar2=None, op0=ADD)
            srh_p = P.tile([D, 2], f32, tag=f"srhp{l}")
            for h in range(2):
                scr = scrp.tile([128, 2 * JH], bf16, tag="scr_act")
                nc.scalar.activation(out=scr[:, 0:JH],
                                     in_=hbT[:, h * JH:(h + 1) * JH],
                                     func=RELU, accum_out=srh_p[:, h:h + 1])
            srh = P.tile([D, 1], f32, tag=f"srh{l}")
            nc.vector.tensor_tensor(out=srh[:], in0=srh_p[:, 0:1],
                                    in1=srh_p[:, 1:2], op=ADD)
            rhb_raw = P.tile([128, NJT, D], bf16, tag=f"rhbraw{l}")
            for jt in range(NJT):
                nc.sync.dma_start(out=rhb_raw[:, jt, :],
                                  in_=hbT[:, jt * 128:(jt + 1) * 128],
                                  transpose=True)
            reluhb = P.tile([128, NJT, D], bf16, tag=f"reluhb{l}")
            nc.vector.tensor_scalar(out=reluhb[:], in0=rhb_raw[:],
                                    scalar1=0.0, scalar2=None, op0=MAX)
            return hbT, srh, reluhb

        def layer(l, xT_l, xownT_l):
            hbT, srh, reluhb = hb_prep(l, xT_l)

            # term1: separate accumulators per consumer engine (avoids
            # cross-engine WAW serialization on a shared tile)
            msgA = P.tile([D, SH], f32, tag=f"msgA{l}")
            msgD = P.tile([D, SH], f32, tag=f"msgD{l}")
            nc.gpsimd.memset(msgA[:], 0.0)
            nc.gpsimd.memset(msgD[:], 0.0)
            for g in range(NG):
                use_act = (g % 2) == 0
                pss = []
                for t in range(4):
                    if use_act:
                        ps = psumA.tile([128, 2 * JH], f32, tag="actps")
                    else:
                        ps = psumD.tile([128, 2 * JH], f32, tag="dveps")
                    pss.append(ps)
                if use_act:
                    for t in range(4):
                        for h in range(2):
                            nc.tensor.matmul(
                                out=pss[t][:, h * JH:(h + 1) * JH],
                                lhsT=ident[:],
                                rhs=hbT[:, h * JH:(h + 1) * JH],
                                start=True, stop=False)
                for t in range(4):
                    for h in range(2):
                        nc.tensor.matmul(
                            out=pss[t][:, h * JH:(h + 1) * JH],
                            lhsT=WeT_rep[l][32 * t:32 * (t + 1), :],
                            rhs=me[32 * t:32 * (t + 1), g, 4 * h:4 * (h + 1), :],
                            start=not use_act, stop=True,
                            tile_position=(32 * t, 0))
                for t in range(4):
                    idx = 4 * g + t
                    if use_act:
                        scr = scrp.tile([128, 2 * JH], bf16, tag="scr_act")
                        nc.scalar.activation(
                            out=scr[:], in_=pss[t][:], func=RELU,
                            accum_out=msgA[:, idx:idx + 1])
                    else:
                        scr = scrd.tile([128, 2 * JH], bf16, tag="scr_dve")
                        nc.vector._custom_dve(
                            relu_add_reduce, out=scr[:], in0=pss[t][:],
                            in1=hbT[:], accum_out=msgD[:, idx:idx + 1])

            # term3: corr[d, i] = sum_j reluhb[j, d] * adjT[j, i]
            ps_corr = psumA.tile([D, SH], f32, tag="actps")
            for jt in range(NJT):
                nc.tensor.matmul(out=ps_corr[:], lhsT=reluhb[:, jt, :],
                                 rhs=adjT[:, jt, :],
                                 start=(jt == 0), stop=(jt == NJT - 1))

            # z = (1+eps)*x_own + msg   (all [d, i_own])
            zm = P.tile([D, SH], f32, tag=f"zm{l}")
            nc.vector.scalar_tensor_tensor(out=zm[:], in0=msgA[:],
                                           scalar=srh[:], in1=msgD[:],
                                           op0=SUB, op1=ADD)
            zt = P.tile([D, SH], f32, tag=f"zt{l}")
            nc.vector.tensor_tensor(out=zt[:], in0=zm[:], in1=ps_corr[:],
                                    op=ADD)
            z_bf = P.tile([D, SH], bf16, tag=f"zbf{l}")
            nc.vector.scalar_tensor_tensor(out=z_bf[:], in0=xownT_l[:],
                                           scalar=opse[:], in1=zt[:],
                                           op0=MULT, op1=ADD)

            # h = relu(Wn @ z + bn) -> [d_out, i_own]
            ps_h = psumA.tile([D, SH], f32, tag="actps")
            nc.tensor.matmul(out=ps_h[:], lhsT=WnT[l][:], rhs=z_bf[:],
                             start=True, stop=True)
            hT = P.tile([D, SH], f32, tag=f"hT{l}")
            nc.scalar.activation(out=hT[:], in_=ps_h[:], func=RELU, bias=bn[l][:])
            return hT

        # ---------------- layer 0 ----------------
        if mode == "edges":
            ps_e = psumA.tile([D, SH], f32, tag="actps")
            nc.tensor.matmul(out=ps_e[:], lhsT=ident[:], rhs=me[:, 0, 0, :],
                             start=True, stop=True)
            h2T = P.tile([D, SH], f32, tag="hTdbg")
            nc.scalar.copy(h2T[:], ps_e[:])
        else:
            h1T = layer(0, xT0, xownT)

        if mode == "l1":
            h2T = h1T
        elif mode == "nocc":
            h2T = layer(1, xT0, h1T)
        elif mode == "x4":
            h = layer(1, xT0, h1T)
            h = layer(0, xT0, h)
            h2T = layer(1, xT0, h)
        elif mode == "full":
            # ------------- allgather updated node features -------------
            h1T_bf = P.tile([D, SH], bf16)
            nc.vector.tensor_scalar(out=h1T_bf[:], in0=h1T[:], scalar1=0.0,
                                    scalar2=None, op0=ADD)
            ps_t = psumA.tile([SH, D], bf16, tag="actps")
            nc.tensor.transpose(ps_t[:], h1T_bf[:], ident[:])
            h1_own = P.tile([SH, D], f32)
            nc.scalar.copy(h1_own[:], ps_t[:])

            gin = dramp.tile([SH, D], f32)
            gout = dramp.tile([N, D], f32)
            nc.gpsimd.dma_start(out=gin[:], in_=h1_own[:])
            nc.gpsimd.collective_compute(
                "AllGather", mybir.AluOpType.bypass,
                replica_groups=[list(range(NC))],
                ins=[gin[:].opt()], outs=[gout[:].opt()])

            # x1T [c, j] bf16 from gathered [N, D] f32: cast + xbar-transpose
            x1b = P.tile([128, NJT, D], bf16)
            nc.gpsimd.dma_start(
                out=x1b[:], in_=gout[:].rearrange("(jt p) d -> p jt d", p=128))
            x1T = P.tile([D, N], bf16)
            for jt in range(NJT):
                nc.sync.dma_start(out=x1T[:, jt * 128:(jt + 1) * 128],
                                  in_=x1b[:, jt, :], transpose=True)

            # ---------------- layer 1 ----------------
            h2T = layer(1, x1T, h1T)

        # ---------------- output ----------------
        h2T_bf = P.tile([D, SH], bf16)
        nc.vector.tensor_scalar(out=h2T_bf[:], in0=h2T[:], scalar1=0.0,
                                scalar2=None, op0=ADD)
        ps_o = psumA.tile([SH, D], bf16, tag="actps")
        nc.tensor.transpose(ps_o[:], h2T_bf[:], ident[:])
        h2_own = P.tile([SH, D], f32)
        nc.scalar.copy(h2_own[:], ps_o[:])
        nc.sync.dma_start(out=out_d[:], in_=h2_own[:])

    nc.compile()
    return nc


def _host_inputs(inputs):
    """Build the 8 per-core input maps from full inputs (host-side staging)."""
    adj = np.ascontiguousarray(np.asarray(inputs["adj"], np.float32))
    nodes = np.ascontiguousarray(np.asarray(inputs["nodes"], np.float32))
    edges = np.ascontiguousarray(np.asarray(inputs["edges"], np.float32))
    eps = float(np.asarray(inputs["eps"], np.float32).reshape(-1)[0])
    com = {
        "xT": np.ascontiguousarray(nodes.T),
        "opse": np.full((128, 1), 1.0 + eps, np.float32),
    }
    Wne = [np.asarray(inputs["Wne0"], np.float32),
           np.asarray(inputs["Wne1"], np.float32)]
    for l in range(2):
        com[f"WeT{l}"] = np.ascontiguousarray(Wne[l][:, D:D + E].T)
        com[f"WnodeT{l}"] = np.ascontiguousarray(Wne[l][:, :D].T)
        com[f"WnT{l}"] = np.ascontiguousarray(
            np.asarray(inputs[f"Wn{l}"], np.float32).T)
        com[f"bne{l}"] = np.ascontiguousarray(
            np.asarray(inputs[f"bne{l}"], np.float32).reshape(D, 1))
        com[f"bn{l}"] = np.ascontiguousarray(
            np.asarray(inputs[f"bn{l}"], np.float32).reshape(D, 1))
    maps = []
    for c in range(NC):
        sl = slice(SH * c, SH * (c + 1))
        m = dict(com)
        m["edges_sh"] = edges[sl]
        m["adjT_sh"] = np.ascontiguousarray(adj[sl].T)
        m["x_ownT"] = np.ascontiguousarray(nodes[sl].T)
        maps.append(m)
    return maps


def _get_runner():
    """Build (once) a cached jit(shard_map) callable over the compiled module."""
    if "runner" in _cache:
        return _cache["runner"]
    import jax
    from jax.sharding import Mesh, PartitionSpec, NamedSharding
    from jax.experimental.shard_map import shard_map
    import concourse.mybir as mybir
    from concourse import bass2jax
    from concourse.bass2jax import _bass_exec_p, partition_id_tensor

    if "nc" not in _cache:
        _cache["nc"] = _build_nc()
    nc = _cache["nc"]
    bass2jax.install_neuronx_cc_hook()

    in_names, out_names, out_avals, zero_outs = [], [], [], []
    partition_name = nc.partition_id_tensor.name if nc.partition_id_tensor else None
    for alloc in nc.m.functions[0].allocations:
        if not isinstance(alloc, mybir.MemoryLocationSet):
            continue
        name = alloc.memorylocations[0].name
        if alloc.kind == "ExternalInput":
            if name != partition_name:
                in_names.append(name)
        elif alloc.kind == "ExternalOutput":
            shape = list(alloc.tensor_shape)
            dtype = np.dtype(mybir.dt.np(alloc.dtype))
            out_avals.append(jax.core.ShapedArray(shape, dtype))
            out_names.append(name)
            zero_outs.append(np.zeros(shape, dtype))

    n_params = len(in_names)
    all_in_names = list(in_names) + list(out_names)
    if partition_name is not None:
        all_in_names.append(partition_name)

    def _body(*args):
        operands = list(args)
        if partition_name is not None:
            operands.append(partition_id_tensor())
        outs = _bass_exec_p.bind(
            *operands,
            out_avals=tuple(out_avals),
            in_names=tuple(all_in_names),
            out_names=tuple(out_names),
            lowering_input_output_aliases=(),
            sim_require_finite=True,
            sim_require_nnan=True,
            nc=nc,
        )
        return tuple(outs)

    devices = jax.devices()[:NC]
    mesh = Mesh(np.asarray(devices), ("core",))
    n_outs = len(out_names)
    fn = jax.jit(
        shard_map(_body, mesh=mesh,
                  in_specs=(PartitionSpec("core"),) * (n_params + n_outs),
                  out_specs=(PartitionSpec("core"),) * n_outs,
                  check_rep=False),
        keep_unused=True)
    sh = NamedSharding(mesh, PartitionSpec("core"))
    dev_zeros = [
        jax.device_put(np.zeros((NC * z.shape[0], *z.shape[1:]), z.dtype), sh)
        for z in zero_outs
    ]

    def run(maps):
        dev_in = []
        for nm in in_names:
            arrs = [
                jax.device_put(np.asarray(maps[c][nm]), devices[c])
                for c in range(NC)
            ]
            shp = arrs[0].shape
            glob = jax.make_array_from_single_device_arrays(
                (NC * shp[0], *shp[1:]), sh, arrs)
            dev_in.append(glob)
        outs = fn(*dev_in, *dev_zeros)
        oi = out_names.index("out")
        return np.asarray(outs[oi]).reshape(NC, SH, D).reshape(N, D)

    _cache["runner"] = run
    return run


def kernel(**inputs):
    run = _get_runner()
    maps = _host_inputs(inputs)
    return np.ascontiguousarray(run(maps).astype(np.float32))


if __name__ == "__main__":
    _build_nc()
    print("build+compile OK")
